# revision 50
# baseline (speedup 1.0000x reference)
"""AttentionBlock kernel for 8 trn2 NeuronCores.

Reference (B=4, C=1024, D=1024, H=16, HD=64, FF=4096):
    xn = LN(x, g1, b1)
    q/k/v = per-head 64x64 projections of xn  (+ bias)
    a = softmax(q k^T / 8)  per head;  attns = mean_h(a)
    o = a v (heads hstacked);  x_bp = o + x
    out = gelu(LN(x_bp, g2, b2) @ W1 + bias1) @ W2 + bias2 + x_bp
    returns (out, attns)

Sharding: core = b*2 + half  (b in 0..3, half in 0..1). Each core computes
query rows [half*512, half*512+512) of batch b end-to-end (k/v over all 1024
keys are recomputed per core; no cross-core communication). Host transposes
x so the on-device dataflow stays fully transposed ([D, C] layouts); host
transposes out back and concatenates.

Design notes:
  - The whole attention datapath runs in bf16 (x, xn, q/k/v weights and
    activations, scores-matmul operands, exp(scores)); psum accumulation
    stays f32. Measured end-to-end error ~5-8e-3 vs the 2e-2 budget.
  - Z (softmax denominator) is folded into the AV matmul via a shared ones
    column in the packed v tile ([v_even | ones | v_odd] per key tile):
    even head reads lhsT cols [0:65) -> O rows 0:64 + Z at row 64; odd
    head reads a 128-wide window from col 1 ([pad | ones | v_o]) -> Z at
    row 63, O at rows 64:128 (partition-aligned with its xbp rows).
  - attns is not normalized/accumulated on device. Raw exp(scores) (bf16)
    and the 1/Z rows are DMA'd out; the host does p = eT * (1/Z) and the
    mean over heads in f32.
  - bk is dropped entirely: softmax over keys is invariant to the
    (q_i + bq) . bk term, which is constant per query row.
  - W1 fully resident in SBUF (DMA'd behind the x loads, no data deps);
    W2 streamed in 2 halves during phase 2.
  - Engine balance in the head loop: exp on ACT, psum->sbuf copies +
    o/residual on DVE, LN1 normalize on GPSIMD (SBUF-only), LN squares
    on ACT. LN2 d-tile squares are precomputed during the attention tail.
"""
import numpy as np
from contextlib import ExitStack

import concourse.mybir as mybir
import concourse.tile as tile
from concourse import bacc
from concourse.alu_op_type import AluOpType as Op
from concourse.bass_utils import run_bass_kernel_spmd

P = 128
B, C, D = 4, 1024, 1024
H, HD = 16, 64
FF = 4 * D
CQ = C // 2          # query rows per core
NKT = C // P         # 8 key tiles
NDT = D // P         # 8 d tiles
NFT = FF // P        # 32 ff tiles
EPS = 1e-5
SCL = 1.0 / np.sqrt(HD)
VW = 2 * HD + 1      # 129: [v_even(64) | ones(1) | v_odd(64)] per key tile

f32 = mybir.dt.float32
f32r = mybir.dt.float32r
bf = mybir.dt.bfloat16
f16 = mybir.dt.float16
AF = mybir.ActivationFunctionType

_CACHE = {}


def _ln_stats(nc, ps, bcps, sqp, bcp, rowp, rowrp, src_tile, ncols,
              ones_col_b, ones_row_r, sq_tiles=None):
    """Stats for LayerNorm over the partition (D) axis of a transposed
    bf16 [D, ncols] tensor packed as NDT tiles in src_tile [128, NDT*ncols].
    Returns (rstd_bc, nmr_bc) [128, ncols] broadcast tiles (from bcp).
    Squares on ACT unless precomputed sq_tiles are passed; rows on DVE."""
    nh = ncols // 512
    rstd_bc = bcp.tile([P, ncols], f16, tag=f"bc{ncols}", name="rstd_bc")
    nmr_bc = bcp.tile([P, ncols], f16, tag=f"bc{ncols}", name="nmr_bc")
    for hf in range(nh):
        ps_mu = ps.tile([1, 512], f32, tag="bank", name="ps_mu")
        for dt in range(NDT):
            src = src_tile[:, dt * ncols + hf * 512 : dt * ncols + hf * 512 + 512]
            nc.tensor.matmul(ps_mu[:], ones_col_b[:], src,
                             start=(dt == 0), stop=(dt == NDT - 1))
        ps_sq = ps.tile([1, 512], f32, tag="bank", name="ps_sq")
        for dt in range(NDT):
            if sq_tiles is not None:
                sq = sq_tiles[dt]
            else:
                sq = sqp.tile([P, 512], bf, tag="sq512", name="sq")
                s_in = src_tile[:, dt * ncols + hf * 512 : dt * ncols + hf * 512 + 512]
                nc.vector.tensor_tensor(out=sq[:], in0=s_in, in1=s_in,
                                        op=Op.mult)
            nc.tensor.matmul(ps_sq[:], ones_col_b[:], sq[:],
                             start=(dt == 0), stop=(dt == NDT - 1))
        mu = rowp.tile([1, 512], f32, tag="row", name="mu")
        nc.vector.tensor_scalar_mul(mu[:], ps_mu[:], 1.0 / D)
        msq = rowp.tile([1, 512], f32, tag="row", name="msq")
        nc.vector.tensor_scalar_mul(msq[:], ps_sq[:], 1.0 / D)
        mu2 = rowp.tile([1, 512], f32, tag="row", name="mu2")
        nc.vector.tensor_tensor(out=mu2[:], in0=mu[:], in1=mu[:], op=Op.mult)
        var = rowp.tile([1, 512], f32, tag="row", name="var")
        nc.vector.scalar_tensor_tensor(out=var[:], in0=msq[:], scalar=EPS,
                                       in1=mu2[:], op0=Op.add, op1=Op.subtract)
        std = rowp.tile([1, 512], f32, tag="row", name="std")
        nc.scalar.activation(std[:], var[:], AF.Sqrt)
        rstd = rowrp.tile([1, 512], f32r, tag="rowr", name="rstd")
        with nc.allow_low_precision("f32r rows feed f32r broadcast matmuls"):
            nc.vector.reciprocal(rstd[:], std[:])
        nmr = rowrp.tile([1, 512], f32r, tag="rowr", name="nmr")
        nc.vector.scalar_tensor_tensor(out=nmr[:], in0=mu[:], scalar=-1.0,
                                       in1=rstd[:].bitcast(f32), op0=Op.mult, op1=Op.mult)
        for row, bc in ((rstd, rstd_bc), (nmr, nmr_bc)):
            ps_b = bcps.tile([P, 512], f32, tag="bcast", name="ps_b")
            nc.tensor.matmul(ps_b[:], ones_row_r[:], row[:], start=True, stop=True)
            nc.scalar.copy(bc[:, hf * 512 : hf * 512 + 512], ps_b[:])
    return rstd_bc, nmr_bc


def _ln_rows(nc, bcps, bcp, rowp, rowrp, ps_mu, ps_sq, ncols, ones_row_r):
    """Turn pre-accumulated sum/sum-of-squares psum rows into broadcast
    (rstd, -mu*rstd) tiles. Single 512-column block (ncols == 512)."""
    rstd_bc = bcp.tile([P, ncols], f16, tag=f"bc{ncols}", name="rstd_bc")
    nmr_bc = bcp.tile([P, ncols], f16, tag=f"bc{ncols}", name="nmr_bc")
    mu = rowp.tile([1, 512], f32, tag="row", name="mu")
    nc.vector.tensor_scalar_mul(mu[:], ps_mu[:], 1.0 / D)
    msq = rowp.tile([1, 512], f32, tag="row", name="msq")
    nc.vector.tensor_scalar_mul(msq[:], ps_sq[:], 1.0 / D)
    mu2 = rowp.tile([1, 512], f32, tag="row", name="mu2")
    nc.vector.tensor_tensor(out=mu2[:], in0=mu[:], in1=mu[:], op=Op.mult)
    var = rowp.tile([1, 512], f32, tag="row", name="var")
    nc.vector.scalar_tensor_tensor(out=var[:], in0=msq[:], scalar=EPS,
                                   in1=mu2[:], op0=Op.add, op1=Op.subtract)
    std = rowp.tile([1, 512], f32, tag="row", name="std")
    nc.scalar.activation(std[:], var[:], AF.Sqrt)
    rstd = rowrp.tile([1, 512], f32r, tag="rowr", name="rstd")
    with nc.allow_low_precision("f32r rows feed f32r broadcast matmuls"):
        nc.vector.reciprocal(rstd[:], std[:])
    nmr = rowrp.tile([1, 512], f32r, tag="rowr", name="nmr")
    nc.vector.scalar_tensor_tensor(out=nmr[:], in0=mu[:], scalar=-1.0,
                                   in1=rstd[:].bitcast(f32), op0=Op.mult, op1=Op.mult)
    for row, bc in ((rstd, rstd_bc), (nmr, nmr_bc)):
        ps_b = bcps.tile([P, 512], f32, tag="bcast", name="ps_b")
        nc.tensor.matmul(ps_b[:], ones_row_r[:], row[:], start=True, stop=True)
        nc.scalar.copy(bc[:], ps_b[:])
    return rstd_bc, nmr_bc


def _ln_norm_hf(nc, eng, sqp, src_tile, dst_tile, ncols, dt, hf, rstd_bc,
                nmr_bc, g_col, b_col):
    _ln_norm_dt(nc, eng, sqp, src_tile, dst_tile, ncols, dt, rstd_bc, nmr_bc,
                g_col, b_col, only_hf=hf)


def _ln_norm_dt(nc, eng, sqp, src_tile, dst_tile, ncols, dt, rstd_bc, nmr_bc,
                g_col, b_col, only_hf=None):
    """Normalize one bf16 d-tile (all ncols columns) given broadcast stats.
    eng: nc.vector or nc.gpsimd (SBUF-only operands required for gpsimd)."""
    nh = ncols // 512
    for hf in range(nh):
        if only_hf is not None and hf != only_hf:
            continue
        c0 = dt * ncols + hf * 512
        xs = src_tile[:, c0 : c0 + 512]
        t = sqp.tile([P, 512], f16, tag="t512", name="t")
        eng.tensor_tensor(out=t[:], in0=xs,
                          in1=rstd_bc[:, hf * 512 : hf * 512 + 512],
                          op=Op.mult)
        if g_col is None:
            eng.tensor_tensor(
                out=dst_tile[:, c0 : c0 + 512], in0=t[:],
                in1=nmr_bc[:, hf * 512 : hf * 512 + 512], op=Op.add,
            )
        else:
            nb = sqp.tile([P, 512], f16, tag="nb512", name="nb")
            eng.tensor_scalar(out=nb[:],
                              in0=nmr_bc[:, hf * 512 : hf * 512 + 512],
                              scalar1=g_col[:, dt : dt + 1],
                              scalar2=b_col[:, dt : dt + 1],
                              op0=Op.mult, op1=Op.add)
            eng.scalar_tensor_tensor(
                out=dst_tile[:, c0 : c0 + 512], in0=t[:],
                scalar=g_col[:, dt : dt + 1], in1=nb[:], op0=Op.mult, op1=Op.add,
            )


def _build(identity_gb=False):
    nc = bacc.Bacc(None, target_bir_lowering=False)
    dp = nc.declare_dram_parameter
    xt_d = dp("xb_t", [D, C], bf, isOutput=False)
    xq_d = dp("xq_b", [P, NDT * CQ], bf, isOutput=False)
    wq_d = dp("wq_b", [P, H * HD], bf, isOutput=False)
    wk_d = dp("wk_b", [P, H * HD], bf, isOutput=False)
    wv_d = dp("wv_b", [P, H * HD], bf, isOutput=False)
    bq_d = dp("bq_c", [P, H], f32, isOutput=False)
    bvc_d = dp("bv_c", [P, H], f32, isOutput=False)
    g1_d = dp("g1_c", [P, NDT], f32, isOutput=False)
    b1_d = dp("b1_c", [P, NDT], f32, isOutput=False)
    g2_d = dp("g2_c", [P, NDT], f32, isOutput=False)
    b2_d = dp("b2_c", [P, NDT], f32, isOutput=False)
    w1_d = dp("w1f", [P, NFT * D], bf, isOutput=False)
    bias1_d = dp("bias1_c", [P, NFT], f32, isOutput=False)
    w2_d = dp("w2f", [P, NFT * D], bf, isOutput=False)
    bias2_d = dp("bias2_c", [P, NDT], f32, isOutput=False)
    sel_d = dp("sel_c", [HD + 1, 2 * P], bf, isOutput=False)
    outT_d = dp("out_t", [D, CQ], f32, isOutput=True)
    et_d = dp("et", [H * P, NKT * CQ], bf, isOutput=True)
    zinv_d = dp("zinv", [2, H // 2 * CQ], bf, isOutput=True)

    with tile.TileContext(nc) as tc, ExitStack() as ctx:
        const = ctx.enter_context(tc.tile_pool(name="const", bufs=1))
        ps = ctx.enter_context(tc.tile_pool(name="ps", bufs=4, space="PSUM"))
        xbp_pool = ctx.enter_context(tc.tile_pool(name="xbpp", bufs=1))

        # ---- constants ----
        ones_col_f = const.tile([P, 1], f32, name="ones_col_f")
        nc.any.memset(ones_col_f[:], 1.0)
        ones_col_b = const.tile([P, 1], bf, name="ones_col_b")
        nc.vector.tensor_copy(ones_col_b[:], ones_col_f[:])
        ones_row = const.tile([1, P], f32, name="ones_row")
        nc.any.memset(ones_row[:], 1.0)
        ones_row_r = const.tile([1, P], f32r, name="ones_row_r")
        nc.vector.tensor_copy(ones_row_r[:], ones_row[:])

        def col_tile(dram, n, name):
            t = const.tile([P, n], f32, name=name)
            nc.sync.dma_start(out=t[:], in_=dram.ap())
            return t

        if identity_gb:
            gb1 = (None, None)
            gb2 = (None, None)
        else:
            gb1 = (col_tile(g1_d, NDT, "g1c"), col_tile(b1_d, NDT, "b1c"))
            gb2 = (col_tile(g2_d, NDT, "g2c"), col_tile(b2_d, NDT, "b2c"))

        w1_sb = const.tile([P, NFT * D], bf, name="w1_sb")
        xbp = xbp_pool.tile([P, NDT * CQ], bf, name="xbp")
        # LN2 d-tile squares, precomputed during the attention tail
        sq8 = [xbp_pool.tile([P, 512], bf, tag=f"sq8_{dt}", name=f"sq8_{dt}")
               for dt in range(NDT)]

        # ======== phase 1: LN1 + attention heads (interleaved) ========
        with tc.tile_pool(name="p1c", bufs=1) as p1c, \
             tc.tile_pool(name="xn", bufs=1) as xn_pool, \
             tc.tile_pool(name="bcp", bufs=2) as bcp, \
             tc.tile_pool(name="sqp", bufs=2) as sqp, \
             tc.tile_pool(name="xtp", bufs=1) as xt_pool:
            # x loads first: everything at t=0 chains off these (the DMA
            # engine pool is a serial resource in issue order)
            xnkv = xt_pool.tile([P, NDT * C], bf, name="xnkv")
            for dt in range(NDT):
                nc.sync.dma_start(
                    out=xnkv[:, dt * C : (dt + 1) * C],
                    in_=xt_d.ap()[dt * P : (dt + 1) * P, :],
                )
            xq_sb = xn_pool.tile([P, NDT * CQ], bf, name="xq_sb")
            nc.sync.dma_start(out=xq_sb[:], in_=xq_d.ap())
            bias1c = col_tile(bias1_d, NFT, "bias1c")
            bias2c = col_tile(bias2_d, NDT, "bias2c")
            # attention-only constants (freed before the MLP phase)
            bqc = p1c.tile([P, H], f32, name="bqc")
            nc.sync.dma_start(out=bqc[:], in_=bq_d.ap())
            bvc = p1c.tile([P, H], f32, name="bvc")
            nc.sync.dma_start(out=bvc[:], in_=bvc_d.ap())
            wq_sb = p1c.tile([P, H * HD], bf, name="wq_sb")
            wk_sb = p1c.tile([P, H * HD], bf, name="wk_sb")
            wv_sb = p1c.tile([P, H * HD], bf, name="wv_sb")
            for w_sb, w_d in ((wq_sb, wq_d), (wk_sb, wk_d), (wv_sb, wv_d)):
                nc.sync.dma_start(out=w_sb[:], in_=w_d.ap())
            # indicator columns (host constant): broadcast row 64 (even
            # heads) / row 63 (odd heads) of the 1/Z tile to all 128
            # partitions via a K=65 matmul (partition 63 directly violates
            # base-partition rules)
            sel_t = p1c.tile([HD + 1, 2 * P], bf, name="sel_t")
            nc.sync.dma_start(out=sel_t[:], in_=sel_d.ap())
            sels = {HD - 1: sel_t[:, 0:P], HD: sel_t[:, P : 2 * P]}
            # 1/Z rows live at the psum partitions where Z lands: row 64
            # for even heads, row 63 for odd heads; slot pr = pair pr.
            # Zeroed once so the K=65 broadcasts see 0 in unused rows.
            zinv2 = p1c.tile([HD + 1, H // 2 * CQ], bf, name="zinv2")
            nc.any.memset(zinv2[:], 0.0)

            with tc.tile_pool(name="rowp", bufs=6) as rowp, \
                 tc.tile_pool(name="rowrp", bufs=4) as rowrp, \
                 tc.tile_pool(name="bcps", bufs=1, space="PSUM") as bcps:
                kv_stats = _ln_stats(nc, ps, bcps, sqp, bcp, rowp, rowrp,
                                     xnkv, C, ones_col_b, ones_row_r)

            with tc.tile_pool(name="headp", bufs=6) as hp, \
                 tc.tile_pool(name="eTp", bufs=5) as eTp, \
                 tc.tile_pool(name="v1p", bufs=3) as v1p, \
                 tc.tile_pool(name="zmiscp", bufs=3) as zmiscp, \
                 tc.tile_pool(name="pf", bufs=2, space="PSUM") as pf, \
                 tc.tile_pool(name="psOp", bufs=2, space="PSUM") as psO_pool:

                def norm_pair(pr, split=False):
                    # DVE for the latency-critical bootstrap pairs; the
                    # otherwise-idle GPSIMD handles the steady state
                    eng = nc.vector if pr < 2 else nc.gpsimd
                    _ln_norm_dt(nc, eng, sqp, xnkv, xnkv, C, pr,
                                *kv_stats, *gb1)

                def prefetch_nv(pr):
                    """Packed v tile [v_even | ones | v_odd] per key tile;
                    the ones column makes the AV matmul also produce Z."""
                    ht = pr
                    v1 = v1p.tile([P, NKT * VW], bf, tag="v1", name="v1")
                    nc.any.memset(
                        v1[:].rearrange("p (k c) -> p k c", k=NKT)[:, :, HD : HD + 1],
                        1.0)
                    for h in (2 * pr, 2 * pr + 1):
                        hs = (h % 2) * HD
                        off = 0 if h % 2 == 0 else HD + 1
                        ps_v = pf.tile([P, 512], f32, tag="pf", name="ps_v")
                        for kt in range(NKT):
                            nc.tensor.matmul(
                                ps_v[:, kt * HD : (kt + 1) * HD],
                                xnkv[hs : hs + HD, ht * C + kt * P : ht * C + (kt + 1) * P],
                                wv_sb[hs : hs + HD, h * HD : (h + 1) * HD],
                                start=True, stop=True,
                            )
                        nc.vector.tensor_copy(
                            v1[:].rearrange("p (k c) -> p k c", k=NKT)[:, :, off : off + HD],
                            ps_v[:].rearrange("p (k c) -> p k c", k=NKT),
                        )
                    return v1

                def prefetch_qk(pr):
                    ht = pr
                    qTs, kTs = {}, {}
                    for h in (2 * pr, 2 * pr + 1):
                        hs = (h % 2) * HD
                        ps_q = pf.tile([HD, 512], f32, tag="pf", name="ps_q")
                        nc.tensor.matmul(
                            ps_q[:], wq_sb[hs : hs + HD, h * HD : (h + 1) * HD],
                            xnkv[hs : hs + HD, ht * C : ht * C + CQ],
                            start=True, stop=True,
                        )
                        qT = hp.tile([HD, CQ], bf, tag="qT", name="qT")
                        nc.vector.tensor_scalar_add(qT[:], ps_q[:],
                                                    bqc[0:HD, h : h + 1])
                        kT = hp.tile([HD, C], bf, tag="kT", name="kT")
                        for hf in range(2):
                            ps_k = pf.tile([HD, 512], f32, tag="pf", name="ps_k")
                            nc.tensor.matmul(
                                ps_k[:], wk_sb[hs : hs + HD, h * HD : (h + 1) * HD],
                                xnkv[hs : hs + HD, ht * C + hf * 512 : ht * C + hf * 512 + 512],
                                start=True, stop=True,
                            )
                            nc.vector.tensor_copy(kT[:, hf * 512 : hf * 512 + 512],
                                                  ps_k[:])
                        qTs[h], kTs[h] = qT, kT
                    return qTs, kTs

                def head_tail(pr, h, psO):
                    """1/Z broadcast + o normalize + residual for head h of
                    pair pr; emitted one head later so the PE never waits on
                    the reciprocal. The odd head also closes out the pair:
                    LN2 square + stats matmuls for its x_bp d-tile."""
                    even = h % 2 == 0
                    hs = (h % 2) * HD
                    zp = HD if even else HD - 1
                    ors = psO[0:HD, :] if even else psO[HD:P, :]
                    ps_zb = ps.tile([P, CQ], f32, tag="bank", name="ps_zb")
                    nc.tensor.matmul(
                        ps_zb[:], sels[zp],
                        zinv2[:, pr * CQ : (pr + 1) * CQ],
                        start=True, stop=True)
                    zb_sb = zmiscp.tile([P, CQ], bf, tag="zbsb", name="zb_sb")
                    nc.vector.tensor_copy(zb_sb[hs : hs + HD, :],
                                          ps_zb[hs : hs + HD, :])
                    o_sb = zmiscp.tile([P, CQ], f32, tag="osb", name="o_sb")
                    nc.vector.tensor_tensor(out=o_sb[hs : hs + HD, :],
                                            in0=ors,
                                            in1=zb_sb[hs : hs + HD, :],
                                            op=Op.mult)
                    last = pr == H // 2 - 1
                    nc.vector.scalar_tensor_tensor(
                        out=xbp[hs : hs + HD, pr * CQ : (pr + 1) * CQ],
                        in0=o_sb[hs : hs + HD, :],
                        scalar=bvc[hs : hs + HD, h : h + 1],
                        in1=xq_sb[hs : hs + HD, pr * CQ : (pr + 1) * CQ],
                        op0=Op.add, op1=Op.add,
                    )
                    if not even:
                        xslice = xbp[:, pr * CQ : (pr + 1) * CQ]
                        nc.gpsimd.tensor_tensor(out=sq8[pr][:], in0=xslice,
                                                in1=xslice, op=Op.mult)

                norm_pair(0, split=True)
                norm_pair(1, split=True)
                qks = {0: prefetch_qk(0), 1: prefetch_qk(1)}
                v1s = {0: prefetch_nv(0)}
                pending = []
                qw1 = NFT * D // 4
                for pr in range(H // 2):
                    if 2 <= pr < 6:
                        # stage a quarter of W1 between the eT exports
                        i = pr - 2
                        nc.sync.dma_start(
                            out=w1_sb[:, i * qw1 : (i + 1) * qw1],
                            in_=w1_d.ap()[:, i * qw1 : (i + 1) * qw1])
                    v1 = v1s.pop(pr)
                    qTs, kTs = qks.pop(pr)
                    for h in (2 * pr, 2 * pr + 1):
                        even = h % 2 == 0
                        qT, kT = qTs[h], kTs[h]
                        eT = eTp.tile([P, NKT * CQ], bf, tag="eT", name="eT")
                        for kt in range(NKT):
                            ps_st = ps.tile([P, CQ], f32, tag="bank", name="ps_st")
                            nc.tensor.matmul(
                                ps_st[:], kT[:, kt * P : (kt + 1) * P], qT[:],
                                start=True, stop=True,
                            )
                            nc.scalar.activation(eT[:, kt * CQ : (kt + 1) * CQ],
                                                 ps_st[:], AF.Exp, scale=SCL)
                        if pending:
                            head_tail(*pending.pop(0))
                        # AV + Z in one accumulation chain (see module doc).
                        psO = psO_pool.tile([P, CQ], f32, tag="psO", name="ps_o")
                        o_dst = psO[0 : HD + 1, :] if even else psO[:]
                        voff, vw = (0, HD + 1) if even else (1, P)
                        for kt in range(NKT):
                            nc.tensor.matmul(
                                o_dst, v1[:, kt * VW + voff : kt * VW + voff + vw],
                                eT[:, kt * CQ : (kt + 1) * CQ],
                                start=(kt == 0), stop=(kt == NKT - 1),
                            )
                        # 1/Z row, kept at Z's own psum partition (64 even,
                        # 63 odd). Engines need partition bases that are
                        # multiples of 32, so the odd head computes over rows
                        # 32:64 - rows 32:62 are junk 1/garbage that the
                        # indicator-masked broadcast multiplies by zero.
                        zp = HD if even else HD - 1
                        lo = HD if even else HD // 2
                        with nc.allow_low_precision("bf16 1/Z rows"):
                            nc.vector.reciprocal(
                                zinv2[lo : zp + 1, pr * CQ : (pr + 1) * CQ],
                                psO[lo : zp + 1, :])
                        nc.sync.dma_start(
                            out=et_d.ap()[h * P : (h + 1) * P, :],
                            in_=eT[:])
                        pending.append((pr, h, psO))
                        # stagger prefetch: norm + v after head A; q/k (two
                        # pairs ahead) after head B
                        if even:
                            if pr + 2 < H // 2:
                                norm_pair(pr + 2)
                            if pr + 1 < H // 2:
                                v1s[pr + 1] = prefetch_nv(pr + 1)
                        elif pr + 2 < H // 2:
                            qks[pr + 2] = prefetch_qk(pr + 2)
                for args in pending:
                    head_tail(*args)
            nc.sync.dma_start(out=zinv_d.ap(), in_=zinv2[HD - 1 : HD + 1, :])

        # ======== phase 2: LN2 + MLP ========
        with tc.tile_pool(name="ln2p", bufs=1) as ln2_pool, \
             tc.tile_pool(name="mlp", bufs=1) as mlp_pool, \
             tc.tile_pool(name="w2p", bufs=2) as w2p, \
             tc.tile_pool(name="outp", bufs=1) as outp:
            ln2 = ln2_pool.tile([P, NDT * CQ], bf, name="ln2")
            with tc.tile_pool(name="sqp2", bufs=4) as sqp, \
                 tc.tile_pool(name="bcp2", bufs=2) as bcp, \
                 tc.tile_pool(name="rowp2", bufs=5) as rowp, \
                 tc.tile_pool(name="rowrp2", bufs=2) as rowrp, \
                 tc.tile_pool(name="bcps2", bufs=1, space="PSUM") as bcps:
                bp_stats = _ln_stats(nc, ps, bcps, sqp, bcp, rowp, rowrp,
                                     xbp, CQ, ones_col_b, ones_row_r,
                                     sq_tiles=sq8)
                # split normalize across DVE and GPSIMD; W1 consumes d-tiles
                # in ascending order, so DVE (faster) takes the early ones
                for dt in range(NDT):
                    _ln_norm_dt(nc, nc.vector, sqp, xbp, ln2, CQ, dt,
                                *bp_stats, *gb2)
            # W2 streamed in 2 halves, both resident during the dh passes
            w2cs = [w2p.tile([P, NFT * D // 2], bf, tag="w2c", name="w2c")
                    for _ in range(2)]
            for i in range(2):
                nc.sync.dma_start(out=w2cs[i][:],
                                  in_=w2_d.ap()[:, i * NFT * D // 2 : (i + 1) * NFT * D // 2])
            hT = mlp_pool.tile([P, NFT * CQ], bf, name="hT")
            for ft in range(NFT):
                ps_h = ps.tile([P, CQ], f32, tag="bank", name="ps_h")
                for dt in range(NDT):
                    nc.tensor.matmul(
                        ps_h[:], w1_sb[:, ft * D + dt * P : ft * D + (dt + 1) * P],
                        ln2[:, dt * CQ : (dt + 1) * CQ],
                        start=(dt == 0), stop=(dt == NDT - 1),
                    )
                nc.scalar.activation(hT[:, ft * CQ : (ft + 1) * CQ], ps_h[:],
                                     AF.Gelu, bias=bias1c[:, ft : ft + 1])
            psacc = ctx.enter_context(tc.tile_pool(name="psacc", bufs=2, space="PSUM"))
            for dt in range(NDT):
                dh, d4 = dt // 4, dt % 4
                ps_out = psacc.tile([P, CQ], f32, tag="pacc", name="ps_out")
                for ft in range(NFT):
                    w2c = w2cs[ft // (NFT // 2)]
                    fo = (ft % (NFT // 2)) * D
                    nc.tensor.matmul(
                        ps_out[:],
                        w2c[:, fo + dh * 512 + d4 * P : fo + dh * 512 + (d4 + 1) * P],
                        hT[:, ft * CQ : (ft + 1) * CQ],
                        start=(ft == 0), stop=(ft == NFT - 1),
                    )
                ot = outp.tile([P, CQ], f32, tag="ot", name="ot")
                nc.vector.scalar_tensor_tensor(
                    out=ot[:], in0=ps_out[:],
                    scalar=bias2c[:, dt : dt + 1],
                    in1=xbp[:, dt * CQ : (dt + 1) * CQ],
                    op0=Op.add, op1=Op.add,
                )
                nc.sync.dma_start(out=outT_d.ap()[dt * P : (dt + 1) * P, :],
                                  in_=ot[:])
    nc.finalize()
    return nc


def _prep_inputs(x, g1, b1, Wq, bq, Wk, bk, Wv, bv, g2, b2, W1, bias1, W2, bias2):
    f = np.float32
    import ml_dtypes
    bf_t = ml_dtypes.bfloat16

    def _dup(W):
        flat = np.asarray(W, dtype=f).transpose(1, 0, 2).reshape(HD, H * HD)
        return np.ascontiguousarray(np.tile(flat, (2, 1)).astype(bf_t))

    def _cols(v, n):
        return np.ascontiguousarray(np.asarray(v, dtype=f).reshape(n, P).T)

    w1f = np.ascontiguousarray(
        np.asarray(W1, dtype=f).reshape(NDT, P, NFT, P).transpose(1, 2, 0, 3)
        .reshape(P, NFT * D).astype(bf_t)
    )
    w2f = np.ascontiguousarray(
        np.asarray(W2, dtype=f).reshape(NFT, P, D).transpose(1, 0, 2)
        .reshape(P, NFT * D).astype(bf_t)
    )
    sel_c = np.zeros((HD + 1, 2 * P), dtype=bf_t)
    sel_c[HD - 1, 0:P] = 1
    sel_c[HD, P : 2 * P] = 1
    shared = {
        "sel_c": sel_c,
        "wq_b": _dup(Wq), "wk_b": _dup(Wk), "wv_b": _dup(Wv),
        "bq_c": np.ascontiguousarray(np.tile(np.asarray(bq, dtype=f).T, (2, 1))),
        "bv_c": np.ascontiguousarray(np.tile(np.asarray(bv, dtype=f).T, (2, 1))),
        "g1_c": _cols(g1, NDT), "b1_c": _cols(b1, NDT),
        "g2_c": _cols(g2, NDT), "b2_c": _cols(b2, NDT),
        "w1f": w1f,
        "bias1_c": _cols(bias1, NFT),
        "w2f": w2f,
        "bias2_c": _cols(bias2, NDT),
    }
    in_maps = []
    for core in range(8):
        b, half = core // 2, core % 2
        xT = np.ascontiguousarray(
            np.roll(np.asarray(x[b], dtype=f).T, -half * CQ, axis=1))
        m = dict(shared)
        m["xb_t"] = np.ascontiguousarray(xT.astype(bf_t))
        m["xq_b"] = np.ascontiguousarray(
            xT[:, :CQ].reshape(NDT, P, CQ).transpose(1, 0, 2)
            .reshape(P, NDT * CQ).astype(bf_t))
        in_maps.append(m)
    return in_maps


def kernel(**inputs):
    trace = inputs.pop("_trace", False)
    ident = (np.all(np.asarray(inputs["g1"]) == 1) and np.all(np.asarray(inputs["b1"]) == 0)
             and np.all(np.asarray(inputs["g2"]) == 1) and np.all(np.asarray(inputs["b2"]) == 0))
    key = f"nc{int(ident)}"
    if key not in _CACHE:
        _CACHE[key] = _build(identity_gb=bool(ident))
    nc = _CACHE[key]
    in_maps = _prep_inputs(**inputs)
    res = None
    last_err = None
    for attempt in range(3):
        try:
            res = run_bass_kernel_spmd(nc, in_maps, list(range(8)), trace=trace)
            break
        except Exception as e:  # transient device wedge: retry
            last_err = e
            import time as _time
            _time.sleep(2.0)
    if res is None:
        raise last_err
    out = np.empty((B, C, D), np.float32)
    attns = np.empty((B, C, C), np.float32)
    for core in range(8):
        b, half = core // 2, core % 2
        r = res.results[core]
        out[b, half * CQ : (half + 1) * CQ, :] = r["out_t"].T
        # host-side softmax normalization + mean over heads. Z comes from
        # summing the exported eT itself, so numerator and denominator share
        # the same roundings and the ratio errors largely cancel.
        e = np.asarray(r["et"]).astype(np.float32).reshape(H, P, NKT, CQ)
        z = e.sum(axis=(1, 2), keepdims=True)           # [H, 1, 1, CQ]
        a = (e / z).mean(axis=0)                        # [kp, kt, q]
        a = a.transpose(2, 1, 0).reshape(CQ, C)         # [q, k'], k'=kt*128+kp
        attns[b, half * CQ : (half + 1) * CQ, :] = np.roll(a, half * CQ, axis=1)
    if trace:
        _CACHE["last_result"] = res
    return (out, attns)
